# revision 10
# baseline (speedup 1.0000x reference)
"""Fused multi-head attention (qkv + RoPE + softmax + proj) for TRN2, 8 cores.

Sharding: core c -> batch b=c//2, head group hg=c%2 (8 of 16 heads).
Data-parallel over B (4), 2-way tensor-parallel over heads.
Host unshard: out[b] = partial[2b] + partial[2b+1] + b_proj.

Per-core device program (matmul inputs in float32r: 1 cycle/row on PE
vs 4 for plain fp32; PSUM accumulation stays fp32):
  phase 1: qT/kT = (Wq/Wk @ x^T) with RoPE applied via a signed
           half-rotation permutation matmul; v in natural [n, dv] layout.
  phase 2: per head pair: sT[k,q] = kT-slices.T @ qT (scores transposed,
           K=64 row-packed pairs), p = exp(sT/8) on ScalarE over a
           [128, 4, 512] psum tile (no max-subtraction: |s| < 8),
           oT[dv,q] = v-slices.T @ p accumulated in PSUM; the softmax
           denominator rides along as a col-packed ones-row matmul in
           its own psum bank (HW start=True clears the whole bank, so
           every accumulation group owns a bank); o is normalized by a
           gpsimd-broadcasted reciprocal of the denominator.
  phase 3: partial proj = aoT-slices.T @ WpT slices, DMA to DRAM.
"""

import sys

if "/opt/trn_rl_repo" not in sys.path:
    sys.path.insert(0, "/opt/trn_rl_repo")

import numpy as np
from contextlib import ExitStack

B, N, C, H, D = 4, 2048, 1024, 16, 64
NCORES = 8
P = 128
DH = 512          # per-core head channels (8 heads x 64)
CT = C // P       # 8 contraction tiles for qkv
DHT = DH // P     # 4 partition tiles of qT/kT/aoT
NT = N // P       # 16 n tiles
NCH = N // 512    # 4 n chunks of 512
KT = N // P       # 16 key tiles

_CACHE = {}


def _emit(nc, tc, mybir, bass, tile):
    F32 = mybir.dt.float32
    F32R = mybir.dt.float32r   # full-rate matmul dtype (np view: float32)
    Exp = mybir.ActivationFunctionType.Exp

    xT = nc.dram_tensor("xT", [C, N], F32R, kind="ExternalInput").ap()
    wq = nc.dram_tensor("wq", [C, DH], F32R, kind="ExternalInput").ap()
    wk = nc.dram_tensor("wk", [C, DH], F32R, kind="ExternalInput").ap()
    wv = nc.dram_tensor("wv", [C, DH], F32R, kind="ExternalInput").ap()
    wp = nc.dram_tensor("wp", [DH, C], F32R, kind="ExternalInput").ap()
    cos2 = nc.dram_tensor("cos2", [P, N], F32, kind="ExternalInput").ap()
    sin2 = nc.dram_tensor("sin2", [P, N], F32, kind="ExternalInput").ap()
    p2t = nc.dram_tensor("p2t", [P, P], F32R, kind="ExternalInput").ap()
    onesd = nc.dram_tensor("onesd", [P, 8], F32R, kind="ExternalInput").ap()
    out = nc.dram_tensor("out", [N, C], F32, kind="ExternalOutput").ap()

    ctx = ExitStack()
    with ctx:
        consts = ctx.enter_context(tc.tile_pool(name="consts", bufs=1))
        persist = ctx.enter_context(tc.tile_pool(name="persist", bufs=1))

        cos_sb = consts.tile([P, N], F32, tag="cos")
        nc.sync.dma_start(cos_sb, cos2)
        sin_sb = consts.tile([P, N], F32, tag="sin")
        nc.sync.dma_start(sin_sb, sin2)
        p2t_sb = consts.tile([P, P], F32R, tag="p2t")
        nc.sync.dma_start(p2t_sb, p2t)

        qT = [persist.tile([P, N], F32R, tag=f"qT{t}", name=f"qT{t}")
              for t in range(DHT)]
        kTt = [persist.tile([P, N], F32R, tag=f"kT{t}", name=f"kT{t}")
               for t in range(DHT)]
        v_sb = [persist.tile([P, 8 * 65], F32R, tag=f"v{i}", name=f"v{i}")
                for i in range(NT)]
        for i in range(NT):
            ones_cols = bass.AP(
                tensor=v_sb[i].tensor, offset=64,
                ap=[list(v_sb[i].ap[0]), [65, 8]])
            nc.sync.dma_start(ones_cols, onesd)

        # ---------------- phase 1: qkv + rope ----------------
        with tc.tile_pool(name="wqkv", bufs=1) as wpool, \
             tc.tile_pool(name="xt", bufs=12) as xpool, \
             tc.tile_pool(name="p1tmp", bufs=3) as tpool, \
             tc.tile_pool(name="p1ps", bufs=2, space="PSUM") as qk_ps_pool, \
             tc.tile_pool(name="p1ps2", bufs=2, space="PSUM") as rot_ps_pool, \
             tc.tile_pool(name="p1ps3", bufs=2, space="PSUM") as v_ps_pool:
            wq_sb = [wpool.tile([P, DH], F32R, tag=f"wq{i}", name=f"wq{i}")
                     for i in range(CT)]
            wk_sb = [wpool.tile([P, DH], F32R, tag=f"wk{i}", name=f"wk{i}")
                     for i in range(CT)]
            wv_sb = [wpool.tile([P, DH], F32R, tag=f"wv{i}", name=f"wv{i}")
                     for i in range(CT)]
            for i in range(CT):
                sl = slice(i * P, (i + 1) * P)
                nc.sync.dma_start(wq_sb[i], wq[sl, :])
                nc.sync.dma_start(wk_sb[i], wk[sl, :])
                nc.sync.dma_start(wv_sb[i], wv[sl, :])

            for nch in range(NCH):
                nsl = slice(nch * 512, (nch + 1) * 512)
                xs = []
                for kc in range(CT):
                    xt = xpool.tile([P, 512], F32R, tag="x")
                    nc.sync.dma_start(xt, xT[kc * P:(kc + 1) * P, nsl])
                    xs.append(xt)
                for w_sb, dst in ((wq_sb, qT), (wk_sb, kTt)):
                    for t in range(DHT):
                        ps = qk_ps_pool.tile([P, 512], F32, tag="qk_ps")
                        for kc in range(CT):
                            nc.tensor.matmul(
                                ps, w_sb[kc][:, t * P:(t + 1) * P], xs[kc],
                                start=(kc == 0), stop=(kc == CT - 1))
                        raw = tpool.tile([P, 512], F32R, tag="raw")
                        nc.vector.tensor_copy(raw, ps)
                        rot = rot_ps_pool.tile([P, 512], F32, tag="rot_ps")
                        nc.tensor.matmul(rot, p2t_sb, raw, start=True, stop=True)
                        t1 = tpool.tile([P, 512], F32, tag="t1")
                        nc.vector.tensor_mul(t1, raw, cos_sb[:, nsl])
                        t2 = tpool.tile([P, 512], F32, tag="t2")
                        nc.vector.tensor_mul(t2, rot, sin_sb[:, nsl])
                        nc.vector.tensor_add(dst[t][:, nsl], t1, t2)
                for nt4 in range(4):
                    i = nch * 4 + nt4
                    ps = v_ps_pool.tile([P, 512], F32, tag="v_ps")
                    for kc in range(CT):
                        nc.tensor.matmul(
                            ps, xs[kc][:, nt4 * P:(nt4 + 1) * P], wv_sb[kc],
                            start=(kc == 0), stop=(kc == CT - 1))
                    v_view = bass.AP(
                        tensor=v_sb[i].tensor, offset=0,
                        ap=[list(v_sb[i].ap[0]), [65, 8], [1, 64]])
                    nc.vector.tensor_copy(v_view, ps.rearrange(
                        "p (h d) -> p h d", h=8))

        # ---------------- phase 2 + 3 pools ----------------
        with tc.tile_pool(name="p23", bufs=1) as p23:
            aoT = [p23.tile([P, N], F32R, tag=f"aoT{t}", name=f"aoT{t}")
                   for t in range(DHT)]
            wp_sb = [p23.tile([P, C], F32R, tag=f"wp{i}", name=f"wp{i}")
                     for i in range(DHT)]
            for i in range(DHT):
                nc.sync.dma_start(wp_sb[i], wp[i * P:(i + 1) * P, :])

            # ---------------- phase 2: attention ----------------
            attn_ctx = ExitStack()
            epool = attn_ctx.enter_context(tc.tile_pool(name="epool", bufs=2))
            atmp = attn_ctx.enter_context(tc.tile_pool(name="atmp", bufs=3))
            s_ps_pool = attn_ctx.enter_context(
                tc.tile_pool(name="s_ps", bufs=1, space="PSUM"))
            o_ps_pool = attn_ctx.enter_context(
                tc.tile_pool(name="o_ps", bufs=2, space="PSUM"))
            for hp in range(4):          # head pairs (even@part0-63, odd@64-127)
                for qc in range(NCH):
                    qsl = slice(qc * 512, (qc + 1) * 512)
                    o_ps = {}
                    for par in range(2):  # par=0: even head, par=1: odd head
                        o_ps[par] = o_ps_pool.tile([P, 512], F32,
                                                   tag=f"o{par}", name=f"o{par}")
                    for ko in range(8):
                        # one [128, 4, 512] psum tensor: (par, j) -> bank
                        s_ps = s_ps_pool.tile([P, 4, 512], F32, tag="s",
                                              name="s")
                        for j in range(2):
                            ki = ko * 2 + j
                            ksl = slice(ki * P, (ki + 1) * P)
                            for par in range(2):
                                pb = par * 64
                                nc.tensor.matmul(
                                    s_ps[:, par * 2 + j],
                                    kTt[hp][pb:pb + 64, ksl],
                                    qT[hp][pb:pb + 64, qsl],
                                    start=True, stop=True,
                                    tile_position=(pb, 0))
                        e = epool.tile([P, 4, 512], F32R, tag="e", name="e")
                        nc.scalar.activation(e, s_ps, Exp,
                                             scale=float(D) ** -0.5)
                        for j in range(2):
                            ki = ko * 2 + j
                            for par in range(2):
                                h = hp * 2 + par
                                # rows 0-63: attn@v for this head;
                                # row 64: softmax denominator (ones col of v)
                                nc.tensor.matmul(
                                    o_ps[par][0:65, :],
                                    v_sb[ki][:, h * 65:(h + 1) * 65],
                                    e[:, par * 2 + j],
                                    start=(ki == 0), stop=(ki == KT - 1))
                    for par in range(2):
                        pb = par * 64
                        r = atmp.tile([P, 512], F32, tag="r")
                        nc.vector.reciprocal(r[0:1, :], o_ps[par][64:65, :])
                        rb = atmp.tile([P, 512], F32, tag="rb")
                        nc.gpsimd.partition_broadcast(
                            rb[0:64, :], r[0:1, :], channels=64)
                        nc.vector.tensor_mul(
                            aoT[hp][pb:pb + 64, qsl],
                            o_ps[par][0:64, :],
                            rb[0:64, :])
            attn_ctx.close()

            # ---------------- phase 3: output projection ----------------
            with tc.tile_pool(name="proj_ps", bufs=2, space="PSUM") as pps, \
                 tc.tile_pool(name="outp", bufs=3) as opool:
                for nt in range(NT):
                    for fc in range(2):
                        ps = pps.tile([P, 512], F32, tag="p")
                        for ct in range(DHT):
                            nc.tensor.matmul(
                                ps,
                                aoT[ct][:, nt * P:(nt + 1) * P],
                                wp_sb[ct][:, fc * 512:(fc + 1) * 512],
                                start=(ct == 0), stop=(ct == DHT - 1))
                        ob = opool.tile([P, 512], F32, tag="ob")
                        nc.vector.tensor_copy(ob, ps)
                        nc.sync.dma_start(
                            out[nt * P:(nt + 1) * P, fc * 512:(fc + 1) * 512],
                            ob)


def build_nc():
    if "nc" in _CACHE:
        return _CACHE["nc"]
    import concourse.bass as bass
    import concourse.tile as tile
    from concourse import bacc, mybir

    nc = bacc.Bacc("TRN2", target_bir_lowering=False, debug=False,
                   enable_asserts=False, num_devices=NCORES)
    with tile.TileContext(nc) as tc:
        _emit(nc, tc, mybir, bass, tile)
    nc.compile()
    _CACHE["nc"] = nc
    return nc


def make_in_maps(x, rope_cos, rope_sin, w_qkv, w_proj):
    x = np.asarray(x, dtype=np.float32)
    rope_cos = np.asarray(rope_cos, dtype=np.float32)
    rope_sin = np.asarray(rope_sin, dtype=np.float32)
    w_qkv = np.asarray(w_qkv, dtype=np.float32)
    w_proj = np.asarray(w_proj, dtype=np.float32)

    cosT = np.ascontiguousarray(rope_cos.T)           # [64, N]
    cos2 = np.vstack([cosT, cosT])                    # [128, N]
    sinT = np.ascontiguousarray(rope_sin.T)
    sin2 = np.vstack([sinT, sinT])

    # signed half-rotation permutation: rot(q) = P2 @ q (per 64-block)
    R = np.zeros((D, D), dtype=np.float32)
    half = D // 2
    R[np.arange(half), np.arange(half) + half] = -1.0
    R[np.arange(half) + half, np.arange(half)] = 1.0
    P2 = np.zeros((P, P), dtype=np.float32)
    P2[:D, :D] = R
    P2[D:, D:] = R
    p2t = np.ascontiguousarray(P2.T)

    xTs = [np.ascontiguousarray(x[b].T) for b in range(B)]

    in_maps = []
    for core in range(NCORES):
        b = core // 2
        hg = core % 2
        in_maps.append({
            "xT": xTs[b],
            "wq": np.ascontiguousarray(w_qkv[hg * DH:(hg + 1) * DH, :].T),
            "wk": np.ascontiguousarray(w_qkv[C + hg * DH:C + (hg + 1) * DH, :].T),
            "wv": np.ascontiguousarray(
                w_qkv[2 * C + hg * DH:2 * C + (hg + 1) * DH, :].T),
            "wp": np.ascontiguousarray(w_proj[:, hg * DH:(hg + 1) * DH].T),
            "cos2": cos2,
            "sin2": sin2,
            "p2t": p2t,
            "onesd": np.ones((P, 8), dtype=np.float32),
        })
    return in_maps


def kernel(x, rope_cos, rope_sin, w_qkv, w_proj, b_proj, trace=False):
    from concourse.bass_utils import run_bass_kernel_spmd

    nc = build_nc()
    in_maps = make_in_maps(x, rope_cos, rope_sin, w_qkv, w_proj)
    res = run_bass_kernel_spmd(nc, in_maps, core_ids=list(range(NCORES)),
                               trace=trace)
    b_proj = np.asarray(b_proj, dtype=np.float32)
    final = np.empty((B, N, C), dtype=np.float32)
    for b in range(B):
        final[b] = res.results[2 * b]["out"] + res.results[2 * b + 1]["out"] \
            + b_proj
    if trace:
        kernel.last_exec_time_ns = res.exec_time_ns
        kernel.last_results = res
    return final


# revision 11
# speedup vs baseline: 1.4441x; 1.4441x over previous
"""Fused multi-head attention (qkv + RoPE + softmax + proj) for TRN2, 8 cores.

Sharding: core c -> batch b=c//2, head group hg=c%2 (8 of 16 heads).
Data-parallel over B (4), 2-way tensor-parallel over heads.
Host unshard: out[b] = partial[2b] + partial[2b+1] + b_proj.

Per-core device program (matmul inputs in float32r: 1 cycle/row on PE
vs 4 for plain fp32; PSUM accumulation stays fp32):
  phase 1: qT/kT = (Wq/Wk @ x^T) with RoPE applied via a signed
           half-rotation permutation matmul; v in natural [n, dv] layout.
  phase 2: per head pair: sT[k,q] = kT-slices.T @ qT (scores transposed,
           K=64 row-packed pairs), p = exp(sT/8) on ScalarE over a
           [128, 4, 512] psum tile (no max-subtraction: |s| < 8),
           oT[dv,q] = v-slices.T @ p accumulated in PSUM; the softmax
           denominator rides along as a col-packed ones-row matmul in
           its own psum bank (HW start=True clears the whole bank, so
           every accumulation group owns a bank); o is normalized by a
           gpsimd-broadcasted reciprocal of the denominator.
  phase 3: partial proj = aoT-slices.T @ WpT slices, DMA to DRAM.
"""

import sys

if "/opt/trn_rl_repo" not in sys.path:
    sys.path.insert(0, "/opt/trn_rl_repo")

import numpy as np
from contextlib import ExitStack

B, N, C, H, D = 4, 2048, 1024, 16, 64
NCORES = 8
P = 128
DH = 512          # per-core head channels (8 heads x 64)
CT = C // P       # 8 contraction tiles for qkv
DHT = DH // P     # 4 partition tiles of qT/kT/aoT
NT = N // P       # 16 n tiles
NCH = N // 512    # 4 n chunks of 512
KT = N // P       # 16 key tiles

_CACHE = {}


def _emit(nc, tc, mybir, bass, tile):
    F32 = mybir.dt.float32
    F32R = mybir.dt.float32r   # full-rate matmul dtype (np view: float32)
    Exp = mybir.ActivationFunctionType.Exp

    xT = nc.dram_tensor("xT", [C, N], F32R, kind="ExternalInput").ap()
    wq = nc.dram_tensor("wq", [C, DH], F32R, kind="ExternalInput").ap()
    wk = nc.dram_tensor("wk", [C, DH], F32R, kind="ExternalInput").ap()
    wv = nc.dram_tensor("wv", [C, DH], F32R, kind="ExternalInput").ap()
    wp = nc.dram_tensor("wp", [DH, C], F32R, kind="ExternalInput").ap()
    cos2 = nc.dram_tensor("cos2", [P, N], F32, kind="ExternalInput").ap()
    sin2 = nc.dram_tensor("sin2", [P, N], F32, kind="ExternalInput").ap()
    p2t = nc.dram_tensor("p2t", [P, P], F32R, kind="ExternalInput").ap()
    onesd = nc.dram_tensor("onesd", [P, 8], F32R, kind="ExternalInput").ap()
    out = nc.dram_tensor("out", [N, C], F32, kind="ExternalOutput").ap()

    ctx = ExitStack()
    with ctx:
        consts = ctx.enter_context(tc.tile_pool(name="consts", bufs=1))
        persist = ctx.enter_context(tc.tile_pool(name="persist", bufs=1))

        cos_sb = consts.tile([P, N], F32, tag="cos")
        nc.sync.dma_start(cos_sb, cos2)
        sin_sb = consts.tile([P, N], F32, tag="sin")
        nc.sync.dma_start(sin_sb, sin2)
        p2t_sb = consts.tile([P, P], F32R, tag="p2t")
        nc.sync.dma_start(p2t_sb, p2t)

        qT = [persist.tile([P, N], F32R, tag=f"qT{t}", name=f"qT{t}")
              for t in range(DHT)]
        kTt = [persist.tile([P, N], F32R, tag=f"kT{t}", name=f"kT{t}")
               for t in range(DHT)]
        v_sb = [persist.tile([P, 8 * 65], F32R, tag=f"v{i}", name=f"v{i}")
                for i in range(NT)]
        for i in range(NT):
            ones_cols = bass.AP(
                tensor=v_sb[i].tensor, offset=64,
                ap=[list(v_sb[i].ap[0]), [65, 8]])
            nc.sync.dma_start(ones_cols, onesd)

        # ---------------- phase 1: qkv + rope ----------------
        with tc.tile_pool(name="wqkv", bufs=1) as wpool, \
             tc.tile_pool(name="xt", bufs=12) as xpool, \
             tc.tile_pool(name="p1tmp", bufs=3) as tpool, \
             tc.tile_pool(name="p1ps", bufs=2, space="PSUM") as qk_ps_pool, \
             tc.tile_pool(name="p1ps2", bufs=2, space="PSUM") as rot_ps_pool, \
             tc.tile_pool(name="p1ps3", bufs=2, space="PSUM") as v_ps_pool:
            wq_sb = [wpool.tile([P, DH], F32R, tag=f"wq{i}", name=f"wq{i}")
                     for i in range(CT)]
            wk_sb = [wpool.tile([P, DH], F32R, tag=f"wk{i}", name=f"wk{i}")
                     for i in range(CT)]
            wv_sb = [wpool.tile([P, DH], F32R, tag=f"wv{i}", name=f"wv{i}")
                     for i in range(CT)]
            for i in range(CT):
                sl = slice(i * P, (i + 1) * P)
                nc.sync.dma_start(wq_sb[i], wq[sl, :])
                nc.sync.dma_start(wk_sb[i], wk[sl, :])
                nc.sync.dma_start(wv_sb[i], wv[sl, :])

            for nch in range(NCH):
                nsl = slice(nch * 512, (nch + 1) * 512)
                xs = []
                for kc in range(CT):
                    xt = xpool.tile([P, 512], F32R, tag="x")
                    nc.sync.dma_start(xt, xT[kc * P:(kc + 1) * P, nsl])
                    xs.append(xt)
                for w_sb, dst in ((wq_sb, qT), (wk_sb, kTt)):
                    for t in range(DHT):
                        ps = qk_ps_pool.tile([P, 512], F32, tag="qk_ps")
                        for kc in range(CT):
                            nc.tensor.matmul(
                                ps, w_sb[kc][:, t * P:(t + 1) * P], xs[kc],
                                start=(kc == 0), stop=(kc == CT - 1))
                        raw = tpool.tile([P, 512], F32R, tag="raw")
                        nc.scalar.copy(raw, ps)
                        rot = rot_ps_pool.tile([P, 512], F32, tag="rot_ps")
                        nc.tensor.matmul(rot, p2t_sb, raw, start=True, stop=True)
                        t1 = tpool.tile([P, 512], F32, tag="t1")
                        nc.vector.tensor_mul(t1, raw, cos_sb[:, nsl])
                        t2 = tpool.tile([P, 512], F32, tag="t2")
                        nc.vector.tensor_mul(t2, rot, sin_sb[:, nsl])
                        nc.gpsimd.tensor_add(dst[t][:, nsl], t1, t2)
                for nt4 in range(4):
                    i = nch * 4 + nt4
                    ps = v_ps_pool.tile([P, 512], F32, tag="v_ps")
                    for kc in range(CT):
                        nc.tensor.matmul(
                            ps, xs[kc][:, nt4 * P:(nt4 + 1) * P], wv_sb[kc],
                            start=(kc == 0), stop=(kc == CT - 1))
                    v_view = bass.AP(
                        tensor=v_sb[i].tensor, offset=0,
                        ap=[list(v_sb[i].ap[0]), [65, 8], [1, 64]])
                    nc.scalar.copy(v_view, ps.rearrange(
                        "p (h d) -> p h d", h=8))

        # ---------------- phase 2 + 3 pools ----------------
        with tc.tile_pool(name="p23", bufs=1) as p23:
            aoT = [p23.tile([P, N], F32R, tag=f"aoT{t}", name=f"aoT{t}")
                   for t in range(DHT)]
            wp_sb = [p23.tile([P, C], F32R, tag=f"wp{i}", name=f"wp{i}")
                     for i in range(DHT)]
            for i in range(DHT):
                nc.sync.dma_start(wp_sb[i], wp[i * P:(i + 1) * P, :])

            # ---------------- phase 2: attention ----------------
            attn_ctx = ExitStack()
            epool = attn_ctx.enter_context(tc.tile_pool(name="epool", bufs=3))
            atmp = attn_ctx.enter_context(tc.tile_pool(name="atmp", bufs=3))
            s_ps_pool = attn_ctx.enter_context(
                tc.tile_pool(name="s_ps", bufs=1, space="PSUM"))
            o_ps_pool = attn_ctx.enter_context(
                tc.tile_pool(name="o_ps", bufs=2, space="PSUM"))
            for hp in range(4):          # head pairs (even@part0-63, odd@64-127)
                for qc in range(NCH):
                    qsl = slice(qc * 512, (qc + 1) * 512)
                    o_ps = {}
                    for par in range(2):  # par=0: even head, par=1: odd head
                        o_ps[par] = o_ps_pool.tile([P, 512], F32,
                                                   tag=f"o{par}", name=f"o{par}")

                    def emit_av(ki, e):
                        for par in range(2):
                            h = hp * 2 + par
                            # rows 0-63: attn@v; row 64: softmax denominator
                            nc.tensor.matmul(
                                o_ps[par][0:65, :],
                                v_sb[ki][:, h * 65:(h + 1) * 65],
                                e[:, par],
                                start=(ki == 0), stop=(ki == KT - 1))

                    prev = None
                    for ki in range(KT):
                        ksl = slice(ki * P, (ki + 1) * P)
                        s_ps = s_ps_pool.tile([P, 2, 512], F32,
                                              tag=f"s{ki % 2}", name=f"s{ki % 2}")
                        for par in range(2):
                            pb = par * 64
                            nc.tensor.matmul(
                                s_ps[:, par],
                                kTt[hp][pb:pb + 64, ksl],
                                qT[hp][pb:pb + 64, qsl],
                                start=True, stop=True,
                                tile_position=(pb, 0))
                        if prev is not None:
                            emit_av(*prev)
                        e = epool.tile([P, 2, 512], F32R, tag="e", name="e")
                        nc.scalar.activation(e, s_ps, Exp,
                                             scale=float(D) ** -0.5)
                        prev = (ki, e)
                    emit_av(*prev)
                    for par in range(2):
                        pb = par * 64
                        r = atmp.tile([P, 512], F32, tag="r")
                        nc.vector.reciprocal(r[0:1, :], o_ps[par][64:65, :])
                        rb = atmp.tile([P, 512], F32, tag="rb")
                        nc.gpsimd.partition_broadcast(
                            rb[0:64, :], r[0:1, :], channels=64)
                        nc.vector.tensor_mul(
                            aoT[hp][pb:pb + 64, qsl],
                            o_ps[par][0:64, :],
                            rb[0:64, :])
            attn_ctx.close()

            # ---------------- phase 3: output projection ----------------
            with tc.tile_pool(name="proj_ps", bufs=2, space="PSUM") as pps, \
                 tc.tile_pool(name="outp", bufs=3) as opool:
                for nt in range(NT):
                    for fc in range(2):
                        ps = pps.tile([P, 512], F32, tag="p")
                        for ct in range(DHT):
                            nc.tensor.matmul(
                                ps,
                                aoT[ct][:, nt * P:(nt + 1) * P],
                                wp_sb[ct][:, fc * 512:(fc + 1) * 512],
                                start=(ct == 0), stop=(ct == DHT - 1))
                        ob = opool.tile([P, 512], F32, tag="ob")
                        nc.vector.tensor_copy(ob, ps)
                        nc.sync.dma_start(
                            out[nt * P:(nt + 1) * P, fc * 512:(fc + 1) * 512],
                            ob)


def build_nc():
    if "nc" in _CACHE:
        return _CACHE["nc"]
    import concourse.bass as bass
    import concourse.tile as tile
    from concourse import bacc, mybir

    nc = bacc.Bacc("TRN2", target_bir_lowering=False, debug=False,
                   enable_asserts=False, num_devices=NCORES)
    with tile.TileContext(nc) as tc:
        _emit(nc, tc, mybir, bass, tile)
    nc.compile()
    _CACHE["nc"] = nc
    return nc


def make_in_maps(x, rope_cos, rope_sin, w_qkv, w_proj):
    x = np.asarray(x, dtype=np.float32)
    rope_cos = np.asarray(rope_cos, dtype=np.float32)
    rope_sin = np.asarray(rope_sin, dtype=np.float32)
    w_qkv = np.asarray(w_qkv, dtype=np.float32)
    w_proj = np.asarray(w_proj, dtype=np.float32)

    cosT = np.ascontiguousarray(rope_cos.T)           # [64, N]
    cos2 = np.vstack([cosT, cosT])                    # [128, N]
    sinT = np.ascontiguousarray(rope_sin.T)
    sin2 = np.vstack([sinT, sinT])

    # signed half-rotation permutation: rot(q) = P2 @ q (per 64-block)
    R = np.zeros((D, D), dtype=np.float32)
    half = D // 2
    R[np.arange(half), np.arange(half) + half] = -1.0
    R[np.arange(half) + half, np.arange(half)] = 1.0
    P2 = np.zeros((P, P), dtype=np.float32)
    P2[:D, :D] = R
    P2[D:, D:] = R
    p2t = np.ascontiguousarray(P2.T)

    xTs = [np.ascontiguousarray(x[b].T) for b in range(B)]

    in_maps = []
    for core in range(NCORES):
        b = core // 2
        hg = core % 2
        in_maps.append({
            "xT": xTs[b],
            "wq": np.ascontiguousarray(w_qkv[hg * DH:(hg + 1) * DH, :].T),
            "wk": np.ascontiguousarray(w_qkv[C + hg * DH:C + (hg + 1) * DH, :].T),
            "wv": np.ascontiguousarray(
                w_qkv[2 * C + hg * DH:2 * C + (hg + 1) * DH, :].T),
            "wp": np.ascontiguousarray(w_proj[:, hg * DH:(hg + 1) * DH].T),
            "cos2": cos2,
            "sin2": sin2,
            "p2t": p2t,
            "onesd": np.ones((P, 8), dtype=np.float32),
        })
    return in_maps


def kernel(x, rope_cos, rope_sin, w_qkv, w_proj, b_proj, trace=False):
    from concourse.bass_utils import run_bass_kernel_spmd

    nc = build_nc()
    in_maps = make_in_maps(x, rope_cos, rope_sin, w_qkv, w_proj)
    res = run_bass_kernel_spmd(nc, in_maps, core_ids=list(range(NCORES)),
                               trace=trace)
    b_proj = np.asarray(b_proj, dtype=np.float32)
    final = np.empty((B, N, C), dtype=np.float32)
    for b in range(B):
        final[b] = res.results[2 * b]["out"] + res.results[2 * b + 1]["out"] \
            + b_proj
    if trace:
        kernel.last_exec_time_ns = res.exec_time_ns
        kernel.last_results = res
    return final
